# revision 24
# baseline (speedup 1.0000x reference)
"""Trainium2 Bass kernel for AttentionWithRoPE, head-sharded across 8 NeuronCores.

Reference computation (fp32):
    q = (x @ Wq) * Dh^-0.5, rope'd; k = (x @ Wk), rope'd; v = x @ Wv
    out = softmax(q k^T * Dh^-0.5) v ;  final = out @ Wo + bo

Sharding: tensor-parallel over heads. Each core owns 2 of 16 heads and
returns a partial [B*N, D] output the host sums (+ rank-1 correction + bo).

Design (measured HW laws: every matmul instruction costs ~25ns +
0.42ns/output-column regardless of dtype/perf-mode, so fp8 DoubleRow --
2 contraction rows per streamed column -- is a straight 2x; ACT runs exp
at 1 elem/lane/cycle; DVE tensor ops 2-4 elem/lane/cycle):

 * fp8e4 DoubleRow everywhere it is numerically safe: q/k/v projections
   (x8 @ 64*W8, the x64 keeps weights out of fp8-subnormal range; 1/64
   rides the rope factor tensors / the v-copy scale), attn@V, softmax
   denominators (ones x pc), and the out-projection. Scores stay fp16
   (contraction is Dh=128 on full partitions; K=64 DoubleRow measures 2x
   slower per column, so packing dh pairs is pointless).
 * Mean subtraction makes fp8 harmless on the value path: scores are tiny
   (std ~0.07), p = exp(s) ~ 1, and any iid fp8 error passes through
   attention's averaging at full relative size. The device works with
   pc = exp(s) - 1 (fp8 error ~0.16% instead of ~2.3%) end to end:
   op_c = sum pc8 v8, denom = N + sum pc8 (the N/64 enters via a const-8
   start matmul so the DVE reciprocal directly yields 64/denom, which is
   also the otc8 pre-scale), otc8 = fp8(op_c * 64/denom), partial =
   otc8 @ wo8. The missing rank-1 "DC term" outer(rc, (sum_m v) @ Wo) per
   (b, head) is added on the host in float64 from the device's returned
   reciprocals. The 1/Dh score scale rides the exp activation immediate.
 * Engine orchestration: the PE queue is in-order, so phase 2 is
   software-pipelined by one (b, nck, j) step -- attn@V / denominator /
   out-projection matmuls of step it-1 (inputs long ready) are drained
   between the score batches of step it, keeping the PE busy while ACT
   streams exp and DVE computes pc8/reciprocal/otc. The b=1 projections
   (+rope) are folded into the same filler queue and overlap phase 2 of
   b=0. GPSIMD does the softmax reciprocal partition-broadcast.
"""

import os
import sys

for _p in ("/opt/trn_rl_repo", "/root/.axon_site/_ro/trn_rl_repo"):
    if os.path.isdir(_p) and _p not in sys.path:
        sys.path.insert(0, _p)

import numpy as np
import ml_dtypes
from contextlib import ExitStack

import concourse.bass as bass
import concourse.bacc as bacc
import concourse.tile as tile
from concourse import mybir
from concourse.bass_utils import run_bass_kernel_spmd

F8 = mybir.dt.float8e4
F16 = mybir.dt.float16
F32 = mybir.dt.float32
AF = mybir.ActivationFunctionType
DR = mybir.MatmulPerfMode.DoubleRow
NP_F8 = ml_dtypes.float8_e4m3

N_CORES = 8
B, N, D, H, Dh = 2, 2048, 2048, 16, 128
HL = H // N_CORES          # heads per core
DHL = HL * Dh              # 256 local head dims
BN = B * N                 # 4096
DCH = D // 128             # 16 contraction chunks
NBLK = BN // 512           # 8 projection column blocks
MCH = N // 128             # 16 key chunks per sequence
NCK = N // 512             # 4 query chunks per sequence
WSCALE = 64.0              # host pre-scale on W before fp8 quantization
OSCALE = WSCALE * WSCALE   # combined otc8 x wo8 output scale

_CACHE = {}
_PHASE_MARKS = {}
import os as _os
SKIP = frozenset(_os.environ.get("PROBE_SKIP", "").split(","))


def _build_nc(loop_n=1):
    nc = bacc.Bacc(trn_type="TRN2", target_bir_lowering=False, debug=False)

    xt_d = nc.dram_tensor("xt", [D, BN], F8, kind="ExternalInput")
    wq_d = nc.dram_tensor("wq", [D, DHL], F8, kind="ExternalInput")
    wk_d = nc.dram_tensor("wk", [D, DHL], F8, kind="ExternalInput")
    wv_d = nc.dram_tensor("wv", [D, DHL], F8, kind="ExternalInput")
    wo_d = nc.dram_tensor("wo", [DHL, D], F8, kind="ExternalInput")
    rope_d = nc.dram_tensor("rope", [2 * B * 2, 128, N], F16, kind="ExternalInput")
    out_d = nc.dram_tensor("out", [BN, D], F16, kind="ExternalOutput")
    rc_d = nc.dram_tensor("rc", [B, HL * N], F16, kind="ExternalOutput")

    xt_v = xt_d.ap().rearrange("(c p) n -> p c n", p=128)       # [128, 16, 4096]
    w_views = {
        "wq": wq_d.ap().rearrange("(c p) m -> p c m", p=128),   # [128, 16, 256]
        "wk": wk_d.ap().rearrange("(c p) m -> p c m", p=128),
        "wv": wv_d.ap().rearrange("(c p) m -> p c m", p=128),
    }
    wo_v = wo_d.ap().rearrange("(j p) d -> p j d", p=128)       # [128, 2, 2048]
    rope_v = rope_d.ap()                                        # [8, 128, 2048]
    out_v = out_d.ap().rearrange("(cb p) d -> cb p d", p=128)   # [32, 128, 2048]

    with tile.TileContext(nc) as tc:
        with ExitStack() as ctx:
            consts = ctx.enter_context(tc.tile_pool(name="consts", bufs=1))
            qtkt = ctx.enter_context(tc.tile_pool(name="qtkt", bufs=1))
            vres = ctx.enter_context(tc.tile_pool(name="vres", bufs=1))
            xin = ctx.enter_context(tc.tile_pool(name="xin", bufs=2))
            ropein = ctx.enter_context(tc.tile_pool(name="ropein", bufs=2))
            tmps = ctx.enter_context(tc.tile_pool(name="tmps", bufs=3))
            ptile = ctx.enter_context(tc.tile_pool(name="ptile", bufs=3))
            pctile = ctx.enter_context(tc.tile_pool(name="pctile", bufs=18))
            rckeep = ctx.enter_context(tc.tile_pool(name="rckeep", bufs=2))
            rbcp = ctx.enter_context(tc.tile_pool(name="rbcp", bufs=2))
            otbuf = ctx.enter_context(tc.tile_pool(name="otbuf", bufs=2))
            obuf = ctx.enter_context(tc.tile_pool(name="obuf", bufs=6))

            psa = ctx.enter_context(tc.tile_pool(name="psa", bufs=2, space="PSUM"))
            psb = ctx.enter_context(tc.tile_pool(name="psb", bufs=3, space="PSUM"))
            psc = ctx.enter_context(tc.tile_pool(name="psc", bufs=1, space="PSUM"))

            # ---- resident weights / constants ----
            w_sb = {}
            for wname in ("wq", "wk", "wv"):
                w_sb[wname] = consts.tile([128, DCH, DHL], F8, name=wname)

            def _load_w(wname):
                for dq in range(4):
                    nc.sync.dma_start(
                        w_sb[wname][:, dq * 4:(dq + 1) * 4, :],
                        w_views[wname][:, dq * 4:(dq + 1) * 4, :],
                    )
            _load_w("wq")
            wo_sb = consts.tile([128, HL, D], F8, name="wo")
            if loop_n > 1:
                nc.sync.dma_start(wo_sb[:], wo_v)
            # DR denominator constants: stationary 1/64 (so the reciprocal
            # yields 64/denom = the otc8 pre-scale) and a moving 8.0 tile
            # whose start-matmul contributes 256*(1/64)*8 = 32 = N/64.
            ones2 = consts.tile([128, 2, 16], F8, name="ones2")
            nc.vector.memset(ones2[:], 1.0 / WSCALE)
            const8 = consts.tile([128, 2, 512], F8, name="const8")
            nc.vector.memset(const8[:], 8.0)
            swap_mask = [(i + 16) % 32 for i in range(32)]

            qt_sb = qtkt.tile([128, HL, BN], F16, name="qt")
            kt_sb = qtkt.tile([128, HL, BN], F16, name="kt")
            v_sb = vres.tile([128, BN // 128, DHL], F8, name="v")

            # ---- shared filler queue (phase-1 b=1 + phase-2 pipelining) ----
            filler = []          # closures: deferred PE-centric work
            post_q = []          # post-chain closures

            def drain(n):
                for _ in range(n):
                    if filler:
                        filler.pop(0)()

            # ---- phase 1: projections + rope ----
            import contextlib
            loop_cm = tc.For_i(0, loop_n, 1) if loop_n > 1 else contextlib.nullcontext()
            with loop_cm:
              def emit_blk(blk):
                  closures = []
                  b = blk // (NBLK // B)
                  c0 = (blk % (NBLK // B)) * 512
                  xblk = xin.tile([128, DCH, 512], F8, name="xblk")
                  rblk = ropein.tile([128, 4, 512], F16, name="rblk")

                  def _dmas():
                      for dq in range(4):
                          nc.sync.dma_start(
                              xblk[:, dq * 4:(dq + 1) * 4, :],
                              xt_v[:, dq * 4:(dq + 1) * 4, blk * 512:(blk + 1) * 512],
                          )
                      nc.sync.dma_start(
                          rblk[:], rope_v[4 * b:4 * b + 4, :, c0:c0 + 512].rearrange("r p n -> p r n")
                      )
                      if blk == 0:
                          _load_w("wk")
                          _load_w("wv")
                  closures.append(_dmas)

                  for wname, dst_sb, ra, rb_ in (
                      ("wq", qt_sb, 0, 1),
                      ("wk", kt_sb, 2, 3),
                  ):
                      for j in range(HL):
                          def _proj(wname=wname, dst_sb=dst_sb, ra=ra, rb_=rb_, j=j):
                              ps = psb.tile([128, 512], F32, name="pb")
                              for dc2 in range(DCH // 2):
                                  nc.tensor.matmul(
                                      ps[:],
                                      w_sb[wname][:, 2 * dc2:2 * dc2 + 2, j * 128:(j + 1) * 128],
                                      xblk[:, 2 * dc2:2 * dc2 + 2, :],
                                      start=(dc2 == 0),
                                      stop=(dc2 == DCH // 2 - 1),
                                      perf_mode=DR,
                                  )
                              raw = tmps.tile([128, 512], F16, name="raw")
                              nc.scalar.copy(raw[:], ps[:])
                              t2 = tmps.tile([128, 512], F16, name="t2")
                              nc.vector.stream_shuffle(t2[:], raw[:], swap_mask)
                              nc.vector.tensor_mul(t2[:], t2[:], rblk[:, rb_, :])
                              nc.vector.tensor_mul(raw[:], raw[:], rblk[:, ra, :])
                              nc.vector.tensor_add(
                                  dst_sb[:, j, blk * 512:(blk + 1) * 512], raw[:], t2[:]
                              )
                          closures.append(_proj)

                  for mc in range(4):
                      def _vproj(mc=mc):
                          psv = psb.tile([128, DHL], F32, name="pb")
                          for dc2 in range(DCH // 2):
                              nc.tensor.matmul(
                                  psv[:],
                                  xblk[:, 2 * dc2:2 * dc2 + 2, mc * 128:(mc + 1) * 128],
                                  w_sb["wv"][:, 2 * dc2:2 * dc2 + 2, :],
                                  start=(dc2 == 0),
                                  stop=(dc2 == DCH // 2 - 1),
                                  perf_mode=DR,
                              )
                          nc.scalar.mul(v_sb[:, blk * 4 + mc, :], psv[:], 1.0 / WSCALE)
                      closures.append(_vproj)
                  return closures

              # b=0 blocks run up front; b=1 blocks become phase-2 fillers
              for blk in range(NBLK // 2):
                  for c in emit_blk(blk):
                      c()
              for blk in range(NBLK // 2, NBLK):
                  filler.extend(emit_blk(blk))

              if loop_n == 1:
                  nc.sync.dma_start(wo_sb[:], wo_v)
              _PHASE_MARKS['end_phase1'] = int(nc.get_next_instruction_name()[2:])
              # ---- phase 2+3: software-pipelined by one (b, nck, j) step.
              # The PE queue is in-order, so attn@V / softmax-post /
              # out-projection for step it-1 (whose pc8 tiles are long
              # ready) are emitted between the score batches of step it;
              # the exp stream on ACT then never starves.
              ob_rr = [0]
              its = [(b, nck, j) for b in range(B) for nck in range(NCK)
                     for j in range(HL)]
              otcs, rcks = {}, {}
              state = {}

              def emit_scores(idx):
                  b, nck, j = its[idx]
                  if (nck, j) == (0, 0):
                      otcs[b] = otbuf.tile([128, HL, N], F8, name="otc")
                      rcks[b] = rckeep.tile([1, HL * N], F16, name="rck")
                  nq0 = b * N + nck * 512
                  pcs = []
                  for mc2 in range(MCH // 2):
                      drain(4 if mc2 else 3)
                      sp = psa.tile([128, 1024], F32, name="pp")
                      for half in range(2):
                          mc = 2 * mc2 + half
                          m0 = b * N + mc * 128
                          nc.tensor.matmul(
                              sp[:, half * 512:(half + 1) * 512],
                              kt_sb[:, j, m0:m0 + 128],
                              qt_sb[:, j, nq0:nq0 + 512],
                              start=True,
                              stop=True,
                          )
                      pt = ptile.tile([128, 1024], F16, name="pt")
                      nc.scalar.activation(pt[:], sp[:], AF.Exp, scale=1.0 / Dh)
                      pc = pctile.tile([128, 1024], F8, name="pc")
                      nc.vector.tensor_scalar_add(pc[:], pt[:], -1.0)
                      pcs.append(pc)
                  state[idx] = pcs
                  while filler:
                      drain(1)
                  if post_q:
                      post_q.pop(0)()

              def emit_attn(idx):
                  b, nck, j = its[idx]
                  pcs = state.pop(idx)
                  op = psb.tile([128, 512], F32, name="pb")
                  dps = psc.tile([1, 512], F32, name="pc")

                  def _dps0():
                      nc.tensor.matmul(
                          dps[:], ones2[:, :, 0:1], const8[:],
                          start=True, stop=False, perf_mode=DR,
                      )
                  filler.append(_dps0)
                  for mc2 in range(MCH // 2):
                      def _pair(mc2=mc2):
                          pcv = pcs[mc2][:].rearrange("p (two n) -> p two n", two=2)
                          nc.tensor.matmul(
                              op[:],
                              v_sb[:, b * MCH + 2 * mc2:b * MCH + 2 * mc2 + 2, j * 128:(j + 1) * 128],
                              pcv,
                              start=(mc2 == 0),
                              stop=(mc2 == MCH // 2 - 1),
                              perf_mode=DR,
                          )
                          nc.tensor.matmul(
                              dps[:],
                              ones2[:, :, 0:1],
                              pcv,
                              start=False,
                              stop=(mc2 == MCH // 2 - 1),
                              perf_mode=DR,
                          )
                      filler.append(_pair)
                  state[("od", idx)] = (op, dps)

              def emit_post(idx):
                  def _post():
                      b, nck, j = its[idx]
                      op, dps = state.pop(("od", idx))
                      rck = rcks[b]
                      rcs = rck[:, j * N + nck * 512:j * N + (nck + 1) * 512]
                      with nc.allow_low_precision(
                          reason="rc in f16 costs ~5e-4 rel on the DC term"
                      ):
                          nc.vector.reciprocal(rcs, dps[:])
                      rbc = rbcp.tile([128, 512], F16, name="rbc")
                      nc.gpsimd.partition_broadcast(rbc[:], rcs, channels=128)
                      nc.vector.tensor_mul(
                          otcs[b][:, j, nck * 512:(nck + 1) * 512], op[:], rbc[:]
                      )
                  post_q.append(_post)

              def emit_outproj(idx):
                  b, nck, j = its[idx]
                  for nck2 in range(4):
                      ncol = nck * 4 + nck2
                      cb = b * (N // 128) + ncol
                      for dcol in range(D // 512):
                          def _op(ncol=ncol, cb=cb, dcol=dcol, b=b):
                              otc = otcs[b]
                              ops3 = psb.tile([128, 512], F32, name="pb")
                              nc.tensor.matmul(
                                  ops3[:],
                                  otc[:, :, ncol * 128:(ncol + 1) * 128],
                                  wo_sb[:, :, dcol * 512:(dcol + 1) * 512],
                                  start=True,
                                  stop=True,
                                  perf_mode=DR,
                              )
                              if "ob" in SKIP:
                                  return
                              ob = obuf.tile([128, 512], F16, name="ob")
                              r = ob_rr[0] % 16
                              ob_rr[0] += 1
                              if r < 6:
                                  nc.scalar.copy(ob[:], ops3[:])
                              else:
                                  nc.vector.tensor_copy(ob[:], ops3[:])
                              nc.sync.dma_start(
                                  out_v[cb, :, dcol * 512:(dcol + 1) * 512], ob[:]
                              )
                          filler.append(_op)

              for idx in range(len(its)):
                  # scores(idx) drains attn(idx-1)+outproj(idx-2) fillers and
                  # fires post(idx-1) at its end
                  emit_scores(idx)
                  if idx >= 1:
                      prev = idx - 1
                      b1, nck1, j1 = its[prev]
                      if j1 == HL - 1:
                          emit_outproj(prev)
                      if (nck1, j1) == (NCK - 1, HL - 1):
                          def _rcdma(b1=b1):
                              nc.sync.dma_start(rc_d.ap()[b1:b1 + 1, :], rcks[b1][:])
                          filler.append(_rcdma)
                  emit_attn(idx)
                  emit_post(idx)
              # drain: last attn + its post, then last outproj + rc dma
              last = len(its) - 1
              while filler:
                  drain(1)
              post_q.pop(0)()
              emit_outproj(last)
              b1 = its[last][0]
              nc.sync.dma_start(rc_d.ap()[b1:b1 + 1, :], rcks[b1][:])
              while filler:
                  drain(1)
              _PHASE_MARKS['end'] = int(nc.get_next_instruction_name()[2:])
    nc.compile()
    return nc


# Permutation of the Dh dim: rotation-pair p = (2p, 2p+1) goes to partitions
# (qd*32 + j, qd*32 + 16 + j) with qd = p // 16, j = p % 16, so the
# real<->imag partner swap is a rotate-by-16 within each 32-partition quadrant
# (expressible as a DVE stream_shuffle).
_PERM = np.empty(Dh, dtype=np.int64)
_PAIR = np.empty(Dh, dtype=np.int64)   # rotation-pair index feeding each partition
_SGN = np.empty(Dh, dtype=np.float64)  # sign of the ri factor at each partition
for _qd in range(4):
    for _j in range(16):
        _p = _qd * 16 + _j
        _PERM[_qd * 32 + _j] = 2 * _p
        _PERM[_qd * 32 + 16 + _j] = 2 * _p + 1
        _PAIR[_qd * 32 + _j] = _p
        _PAIR[_qd * 32 + 16 + _j] = _p
        _SGN[_qd * 32 + _j] = -1.0
        _SGN[_qd * 32 + 16 + _j] = 1.0


def _to_f8(a):
    return np.clip(a, -240.0, 240.0).astype(NP_F8)


def _prep_inputs(x, q_rope, k_rope, Wq, Wk, Wv, Wo):
    xt = np.ascontiguousarray(_to_f8(x.reshape(BN, D).T))

    # rope factor tensors: per batch [qrA, qrB, krA, krB], each [128, N].
    # 1/WSCALE removes the x64 weight pre-scale; the 1/Dh score scale is
    # applied later inside the exp activation.
    ropes = []
    for b in range(B):
        for r, scale in ((q_rope[b], 1.0 / WSCALE), (k_rope[b], 1.0 / WSCALE)):
            rr = r[:, 0::2].T * scale   # [64, N], indexed by rotation pair
            ri = r[:, 1::2].T * scale
            ropes.append(rr[_PAIR])                  # A: rr at both partners
            ropes.append(ri[_PAIR] * _SGN[:, None])  # B: -ri at real, +ri at imag
    rope_all = np.ascontiguousarray(np.stack(ropes).astype(np.float16))

    in_maps = []
    for c in range(N_CORES):
        heads = range(HL * c, HL * (c + 1))
        wq_c = np.concatenate(
            [Wq[:, h * Dh:(h + 1) * Dh][:, _PERM] for h in heads], axis=1
        ) * WSCALE
        wk_c = np.concatenate(
            [Wk[:, h * Dh:(h + 1) * Dh][:, _PERM] for h in heads], axis=1
        ) * WSCALE
        wv_c = np.concatenate(
            [Wv[:, h * Dh:(h + 1) * Dh] for h in heads], axis=1
        ) * WSCALE
        wo_c = np.concatenate(
            [Wo[h * Dh:(h + 1) * Dh, :] for h in heads], axis=0
        ) * WSCALE
        in_maps.append(
            {
                "xt": xt,
                "wq": np.ascontiguousarray(_to_f8(wq_c)),
                "wk": np.ascontiguousarray(_to_f8(wk_c)),
                "wv": np.ascontiguousarray(_to_f8(wv_c)),
                "wo": np.ascontiguousarray(_to_f8(wo_c)),
                "rope": rope_all,
            }
        )
    return in_maps


def kernel(x, q_rope, k_rope, Wq, Wk, Wv, Wo, bo, **run_kwargs):
    if "nc" not in _CACHE:
        _CACHE["nc"] = _build_nc()
    nc = _CACHE["nc"]

    in_maps = _prep_inputs(x, q_rope, k_rope, Wq, Wk, Wv, Wo)
    res = run_bass_kernel_spmd(nc, in_maps, core_ids=list(range(N_CORES)), **run_kwargs)

    # host: sum fp8 partials (descaled) + exact rank-1 DC term + bias
    total = np.zeros((BN, D), dtype=np.float32)
    for c in range(N_CORES):
        total += res.results[c]["out"].astype(np.float32)
    total *= 1.0 / OSCALE
    # DC term: sum_m v(m, dh) in float64 (exact), times the device's rc
    v_ref = x.astype(np.float64).reshape(BN, D) @ Wv.astype(np.float64)
    Vsum = v_ref.reshape(B, N, H, Dh).sum(axis=1)          # [B, H, Dh]
    total = total.reshape(B, N, D)
    for c in range(N_CORES):
        # device rc carries the x64 pre-scale
        rc = res.results[c]["rc"].astype(np.float64) / WSCALE   # [B, HL*N]
        for j in range(HL):
            h = HL * c + j
            Wbar = (Vsum[:, h] @ Wo[h * Dh:(h + 1) * Dh].astype(np.float64))
            for b in range(B):
                total[b] += np.outer(
                    rc[b, j * N:(j + 1) * N], Wbar[b]
                ).astype(np.float32)
    total += bo.astype(np.float32)[None, None, :]
    _CACHE["last_res"] = res
    return total
